# revision 5
# baseline (speedup 1.0000x reference)
"""AlignmentAttentionLayer Trainium2 kernel (8 NeuronCores, data-parallel).

Math per batch row b (D=300, L=50):
    M     = tanh(W_y @ Y[b] + (W_h @ h_n[b]) 1_L^T)     [D, L]
    alpha = softmax(w^T M)  over L                       [L]
    r     = Y[b] @ alpha                                 [D]
    out   = tanh(W_p @ r + W_x @ h_n[b])                 [D]

Sharding: batch dim B=16384 split across 8 cores (2048 rows each);
weights replicated. No collectives needed.

Layout strategy per core:
  - Y chunk in SBUF as [e(partitions, 3 subtiles of 128/128/44), b, l],
    cast once to bf16 (PE + DVE run faster on bf16).
  - All D x D weights preloaded transposed [e, d] as bf16.
  - M = tanh(W_y Y + bias) computed per 400-column chunk in PSUM
    ([d_sub, (b,l)]), bias added on DVE, tanh on ACT, output bf16.
  - s = w^T M via PE matmul with lhsT = w replicated to 128 columns,
    so exp(s) lands replicated across all 128 partitions, which makes
    the alpha * Y elementwise product / row-reduction partition-parallel.
  - softmax normalization folded into r: r = (Y @ exp(s)) / sum(exp(s)).
  - h* accumulated in PSUM from 6 matmuls (W_p r + W_x h_n), tanh,
    PE-transposed back to [b, d] and DMA'd out contiguously.
"""

import numpy as np

# ---- constants (hardcoded per problem spec) -------------------------------
B = 16384
D = 300
L = 50
NCORES = 8
BB = B // NCORES          # 2048 batch rows per core
P = 128
SUBS = [(0, 128), (128, 128), (256, 44)]   # subtiles of the 300-dim axis
NB = 64                   # batch rows per outer chunk
NCH = 8                   # inner column-chunks per outer chunk
NI = NB * L // NCH        # 400 (b,l) columns per inner chunk
RI = NI // L              # 8 batch rows per inner chunk


def _build(bb: int):
    import concourse.bass as bass
    import concourse.mybir as mybir
    from concourse.tile import TileContext
    from concourse.masks import make_identity

    f32 = mybir.dt.float32
    bf16 = mybir.dt.bfloat16
    AF = mybir.ActivationFunctionType
    OP = mybir.AluOpType
    AX = mybir.AxisListType

    from contextlib import ExitStack

    nc = bass.Bass("TRN2")
    Y_d = nc.declare_dram_parameter("Y", [bb, D, L], f32, isOutput=False)
    hn_d = nc.declare_dram_parameter("h_n", [bb, D], f32, isOutput=False)
    Wy_d = nc.declare_dram_parameter("W_y", [D, D], f32, isOutput=False)
    Wh_d = nc.declare_dram_parameter("W_h", [D, D], f32, isOutput=False)
    Wp_d = nc.declare_dram_parameter("W_p", [D, D], f32, isOutput=False)
    Wx_d = nc.declare_dram_parameter("W_x", [D, D], f32, isOutput=False)
    w_d = nc.declare_dram_parameter("w", [D], f32, isOutput=False)
    out_d = nc.declare_dram_parameter("out", [bb, D], f32, isOutput=True)

    chunks = bb // NB

    with TileContext(nc) as tc, ExitStack() as ctx:
        const = ctx.enter_context(tc.tile_pool(name="const", bufs=1))
        init = ctx.enter_context(tc.tile_pool(name="init", bufs=2))
        work = ctx.enter_context(tc.tile_pool(name="work", bufs=2))
        inner = ctx.enter_context(tc.tile_pool(name="inner", bufs=3))
        psM = ctx.enter_context(tc.tile_pool(name="psM", bufs=3, space="PSUM"))
        psS = ctx.enter_context(tc.tile_pool(name="psS", bufs=2, space="PSUM"))
        psA = ctx.enter_context(tc.tile_pool(name="psA", bufs=2, space="PSUM"))

        ident = const.tile([P, P], f32, tag="ident")
        make_identity(nc, ident)

        # ---- preload weights, transposed [e, d], bf16 ----
        wTs = {}
        for name, wd in (("wy", Wy_d), ("wh", Wh_d), ("wp", Wp_d), ("wx", Wx_d)):
            wT = const.tile([P, 3, D], bf16, tag=f"{name}T")
            tmp = init.tile([P, 3, D], f32, tag="wtmp")
            with nc.allow_non_contiguous_dma(reason="one-time 300x300 transpose load"):
                for es, (e0, pe) in enumerate(SUBS):
                    nc.sync.dma_start(out=tmp[:pe, es, :], in_=wd[:, e0:e0 + pe].rearrange("d e -> e d"))
            for es, (e0, pe) in enumerate(SUBS):
                nc.gpsimd.tensor_copy(out=wT[:pe, es, :], in_=tmp[:pe, es, :])
            wTs[name] = wT
        wyT, whT, wpT, wxT = wTs["wy"], wTs["wh"], wTs["wp"], wTs["wx"]

        # ---- w replicated to 128 columns: lhsT for the s-matmul ----
        wv = const.tile([P, 3], f32, tag="wv")
        with nc.allow_non_contiguous_dma(reason="one-time 300-elem strided load"):
            for es, (e0, pe) in enumerate(SUBS):
                nc.sync.dma_start(out=wv[:pe, es:es + 1], in_=w_d[e0:e0 + pe, None])
        w_repl = const.tile([P, 3, P], bf16, tag="w_repl")
        for es, (e0, pe) in enumerate(SUBS):
            nc.vector.tensor_copy(out=w_repl[:pe, es, :], in_=wv[:pe, es, None].to_broadcast((pe, P)))

        # ---- main loop over batch chunks ----
        for c in range(chunks):
            b0 = c * NB

            Yf = work.tile([P, 3, NB, L], f32, tag="Yf")
            for es, (e0, pe) in enumerate(SUBS):
                nc.sync.dma_start(
                    out=Yf[:pe, es],
                    in_=Y_d[b0:b0 + NB, e0:e0 + pe, :].rearrange("b e l -> e b l"),
                )
            Yb = work.tile([P, 3, NB, L], bf16, tag="Yb")
            for es, (e0, pe) in enumerate(SUBS):
                nc.gpsimd.tensor_copy(out=Yb[:pe, es], in_=Yf[:pe, es])

            hn = work.tile([NB, D], f32, tag="hn")
            nc.sync.dma_start(out=hn[:], in_=hn_d[b0:b0 + NB, :])
            hnT = work.tile([P, 3, NB], bf16, tag="hnT")
            for es, (e0, pe) in enumerate(SUBS):
                pt = psA.tile([P, P], f32, tag="psa", name="pt")[:, :NB]
                nc.tensor.transpose(pt[:pe, :NB], hn[:, e0:e0 + pe], ident[:NB, :NB])
                nc.scalar.copy(out=hnT[:pe, es, :], in_=pt[:pe, :NB])

            # bias: wh[d, b] = W_h @ h_n^T
            wh = work.tile([P, 3, NB], f32, tag="wh")
            for ds, (d0, pd) in enumerate(SUBS):
                pw = psA.tile([P, P], f32, tag="psa", name="pw")[:, :NB]
                for es, (e0, pe) in enumerate(SUBS):
                    nc.tensor.matmul(pw[:pd, :], whT[:pe, es, d0:d0 + pd], hnT[:pe, es, :],
                                     start=(es == 0), stop=(es == 2))
                nc.scalar.copy(out=wh[:pd, ds, :], in_=pw[:pd, :])

            rT = work.tile([P, 3, NB], f32, tag="rT")
            z = work.tile([P, NB], f32, tag="z")

            for t in range(NCH):
                r0 = t * RI
                Mb = inner.tile([P, 3, NI], bf16, tag="Mb")
                for ds, (d0, pd) in enumerate(SUBS):
                    pm = psM.tile([P, NI], f32, tag="pm")
                    for es, (e0, pe) in enumerate(SUBS):
                        nc.tensor.matmul(
                            pm[:pd, :], wyT[:pe, es, d0:d0 + pd],
                            Yb[:pe, es, r0:r0 + RI, :], start=(es == 0), stop=(es == 2))
                    nc.vector.tensor_tensor(
                        out=Mb[:pd, ds].rearrange("p (b l) -> p b l", l=L),
                        in0=pm[:pd].rearrange("p (b l) -> p b l", l=L),
                        in1=wh[:pd, ds, r0:r0 + RI, None].to_broadcast((pd, RI, L)),
                        op=OP.add)
                    nc.scalar.activation(out=Mb[:pd, ds], in_=Mb[:pd, ds], func=AF.Tanh)

                ps_s = psS.tile([P, NI], f32, tag="ps_s")
                for ds, (d0, pd) in enumerate(SUBS):
                    nc.tensor.matmul(ps_s[:, :], w_repl[:pd, ds, :], Mb[:pd, ds],
                                     start=(ds == 0), stop=(ds == 2))
                alpha = inner.tile([P, NI], bf16, tag="alpha")
                nc.scalar.activation(out=alpha[:], in_=ps_s[:], func=AF.Exp)
                nc.vector.tensor_reduce(
                    out=z[:, r0:r0 + RI],
                    in_=alpha.rearrange("p (b l) -> p b l", l=L),
                    axis=AX.X, op=OP.add)
                for es, (e0, pe) in enumerate(SUBS):
                    prod = inner.tile([P, RI, L], bf16, tag="prod")
                    nc.vector.tensor_mul(
                        out=prod[:pe],
                        in0=Yb[:pe, es, r0:r0 + RI, :],
                        in1=alpha[:pe].rearrange("p (b l) -> p b l", l=L))
                    nc.vector.tensor_reduce(
                        out=rT[:pe, es, r0:r0 + RI], in_=prod[:pe],
                        axis=AX.X, op=OP.add)

            zinv = work.tile([P, NB], f32, tag="zinv")
            nc.vector.reciprocal(zinv[:], z[:])
            rTb = work.tile([P, 3, NB], bf16, tag="rTb")
            nc.vector.tensor_mul(out=rTb[:], in0=rT[:],
                                 in1=zinv[:, None, :].to_broadcast((P, 3, NB)))

            ho = work.tile([NB, D], f32, tag="ho")
            for ds, (d0, pd) in enumerate(SUBS):
                ph = psA.tile([P, P], f32, tag="psa", name="ph")[:, :NB]
                for es, (e0, pe) in enumerate(SUBS):
                    nc.tensor.matmul(ph[:pd, :], wpT[:pe, es, d0:d0 + pd], rTb[:pe, es, :],
                                     start=(es == 0), stop=False)
                for es, (e0, pe) in enumerate(SUBS):
                    nc.tensor.matmul(ph[:pd, :], wxT[:pe, es, d0:d0 + pd], hnT[:pe, es, :],
                                     start=False, stop=(es == 2))
                hs = work.tile([P, NB], f32, tag="hs")
                nc.scalar.activation(out=hs[:pd, :], in_=ph[:pd, :], func=AF.Tanh)
                pt2 = psA.tile([P, P], f32, tag="psa", name="pt2")
                nc.tensor.transpose(pt2[:NB, :pd], hs[:pd, :NB], ident[:pd, :pd])
                nc.scalar.copy(out=ho[:, d0:d0 + pd], in_=pt2[:NB, :pd])

            nc.sync.dma_start(out=out_d[b0:b0 + NB, :], in_=ho[:])

    return nc


_NC_CACHE = {}


def _get_nc(bb: int):
    if bb not in _NC_CACHE:
        import sys, os
        sys.path.insert(0, os.path.dirname(os.path.abspath(__file__)))
        try:
            import tile_patch
            tile_patch.install()
        except Exception:
            pass
        _NC_CACHE[bb] = _build(bb)
    return _NC_CACHE[bb]


def kernel(Y, h_n, W_y, W_h, W_p, W_x, w, _collect=None):
    from concourse.bass_utils import run_bass_kernel_spmd

    Y = np.ascontiguousarray(np.asarray(Y, dtype=np.float32))
    h_n = np.ascontiguousarray(np.asarray(h_n, dtype=np.float32))
    W_y = np.ascontiguousarray(np.asarray(W_y, dtype=np.float32))
    W_h = np.ascontiguousarray(np.asarray(W_h, dtype=np.float32))
    W_p = np.ascontiguousarray(np.asarray(W_p, dtype=np.float32))
    W_x = np.ascontiguousarray(np.asarray(W_x, dtype=np.float32))
    w = np.ascontiguousarray(np.asarray(w, dtype=np.float32))

    bb = Y.shape[0] // NCORES
    nc = _get_nc(bb)
    in_maps = [
        {
            "Y": Y[i * bb:(i + 1) * bb],
            "h_n": h_n[i * bb:(i + 1) * bb],
            "W_y": W_y, "W_h": W_h, "W_p": W_p, "W_x": W_x, "w": w,
        }
        for i in range(NCORES)
    ]
    res = run_bass_kernel_spmd(nc, in_maps, core_ids=list(range(NCORES)))
    if _collect is not None:
        _collect.append(res)
    return np.concatenate([res.results[i]["out"] for i in range(NCORES)], axis=0)
